# revision 62
# baseline (speedup 1.0000x reference)
"""Trainium2 Bass kernel: RMSNorm + QKV + YaRN RoPE + sliding-window GQA attention
with sink logits + output projection + residual.

Sharding: data-parallel over batch (2) x tensor-parallel over KV-head pairs (4).
Each of the 8 cores computes, for one batch element and 2 of the 8 KV heads
(16 of the 64 Q heads), the fused block and a partial output projection.
The host sums the 4 partial projections per batch and adds bias + residual.

v3 precision/PE strategy: all big GEMMs run as fp8e4m3 DoubleRow matmuls with
hi/lo residual splitting, which keeps bf16-level accuracy at 0.75x the PE cost
(cost model: DR = 0.5 cycles/row with 2 contraction tiles per instruction):
  - QKV: x and w split on the host (x = x8hi + x8lo, 64*w = w8hi + w8lo);
    per (m, chunk): 12 hi*hi DR (ktile-paired) + 23 cross DR (slot-paired
    (w8hi_k (x) x8lo_k, w8lo_k (x) x8hi_k)); lo*lo dropped. Epilogue
    rescales by 1/64 and adds bias.
  - Out-projection: attn (bf16) is split on device per (qt, a) tile:
    a8hi = fp8(a), a8lo = fp8(8*(a - a8hi)) via ACT copy + DVE sub + ACT
    scaled copy. Weights host-split (32*w = w8hi + w8lo, plus w8hiD8 =
    fp8(w'/8) for the a8lo slots). 4 hi*hi DR + 8 cross DR per (m, chunk).
  - Additive attention masks (-96 live/masked) and the softmax sink row are
    exact in fp8 and issued as DR matmuls at half cost.
  - Attention QK / PV stay bf16; exp on ACT with scale=0.125 (sm_scale).
  - Output partials stored fp16 (scale 1/32 applied in the epilogue).
"""

import numpy as np
import ml_dtypes

import concourse.bass as bass
import concourse.tile as tile
from concourse import bacc, mybir
from concourse.bass_utils import run_bass_kernel_spmd

# problem constants
B, SEQ, HID = 2, 1024, 2880
NH, NKV, D = 64, 8, 64
KT = 23                # real hidden k-tiles (2944 = 23*128 padded)
KTP = 24               # padded (for hi-hi DR pairing)
HIDP = KT * 128        # 2944
HIDP2 = KTP * 128      # 3072
QKV_M = 10             # 1280 rows per core / 128
OUT_M = KT             # output hidden tiles
OUT_K = 8              # 1024 attn features / 128
NT = SEQ               # tokens per core
CH = 512               # matmul moving chunk
EPS = 1e-5
MASK_NEG = -96.0       # exact in fp8e4m3; exp scale 0.125 -> e^-12

F32 = mybir.dt.float32
F16 = mybir.dt.float16
BF16 = mybir.dt.bfloat16
FP8 = mybir.dt.float8e4
DR = mybir.MatmulPerfMode.DoubleRow

PAIR_SWAP = [i ^ 1 for i in range(32)]
CHASE_M = (8, 9, 0, 1, 2, 3)  # m-slot order inside wqc8


# ---------------------------------------------------------------- device code
def build_nc(timing_mode=False):
    nc = bacc.Bacc("TRN2", target_bir_lowering=False, debug=False)

    big = "Internal" if timing_mode else "ExternalInput"
    # x8 ktile-major: col = k*2048 + s*1024 + t (s: lo,hi)
    x8_d = nc.dram_tensor("x8", [128, KTP * 2 * NT], FP8, kind=big)
    # chase weights m in {8,9,0,1,2,3}: col = k*1536 + j*256 + s*128 + f
    wqc8_d = nc.dram_tensor("wqc8", [128, KTP * 1536], FP8, kind=big)
    # m=4..7 weights: col = k*256 + s*128 + f
    wq8_d = nc.dram_tensor("wq8", [4, 128, KTP * 256], FP8, kind=big)
    bqkv_d = nc.dram_tensor("bqkv", [128, QKV_M], F32, kind="ExternalInput")
    # out-proj weights per m: [hi(8*128) | hiD8(8*128) | lo(8*128)]
    wo8_d = nc.dram_tensor("wo8", [OUT_M, 128, 3 * OUT_K * 128], FP8, kind=big)
    cos_d = nc.dram_tensor("cos128", [128, NT], BF16, kind="ExternalInput")
    sin_d = nc.dram_tensor("sin128", [128, NT], BF16, kind="ExternalInput")
    mask8_d = nc.dram_tensor("mask8", [128, 256], FP8, kind="ExternalInput")
    seld8_d = nc.dram_tensor("seld8", [128, 1024], FP8, kind="ExternalInput")
    sell8_d = nc.dram_tensor("sell8", [128, 1024], FP8, kind="ExternalInput")
    sinksel_d = nc.dram_tensor("sinksel8", [1, 160], FP8, kind="ExternalInput")
    sinkrow_d = nc.dram_tensor("sinkrow8", [1, 2 * 4 * CH], FP8,
                               kind="ExternalInput")
    id_d = nc.dram_tensor("ident64", [128, 64], BF16, kind="ExternalInput")
    out_d = nc.dram_tensor("out_t", [OUT_M, 128, NT], F16,
                           kind="Internal" if timing_mode else "ExternalOutput")
    dummy_d = (nc.dram_tensor("timing_out", [1, 2], F32, kind="ExternalOutput")
               if timing_mode else None)

    mult = mybir.AluOpType.mult
    sub = mybir.AluOpType.subtract

    with tile.TileContext(nc) as tc:
        with tc.tile_pool(name="singles", bufs=1) as singles:
            cos_sb = singles.tile([128, NT], BF16)
            sin_sb = singles.tile([128, NT], BF16)
            bq_sb = singles.tile([128, QKV_M], F32)
            mask8_sb = singles.tile([128, 256], FP8)
            seld8_sb = singles.tile([128, 1024], FP8)
            sell8_sb = singles.tile([128, 1024], FP8)
            sinksel_sb = singles.tile([1, 160], FP8)
            sinkrow_sb = singles.tile([1, 2 * 4 * CH], FP8)
            id_sb = singles.tile([128, 64], BF16)

            q_sb = singles.tile([128, 8 * NT], BF16)
            k_sb = singles.tile([128, NT], BF16)
            v_sb = singles.tile([128, NT], BF16)
            vt_sb = singles.tile([128, 16 * 65], BF16)
            a_sb = singles.tile([128, 8 * NT], BF16)
            a8_sb = singles.tile([128, 2 * 8 * NT], FP8)  # [lo | hi]

            q_v = q_sb.rearrange("p (h t) -> p h t", t=NT)
            a_v = a_sb.rearrange("p (h t) -> p h t", t=NT)
            a8_v = a8_sb.rearrange("p (s h t) -> p s h t", s=2, t=NT)
            mask8_v = mask8_sb.rearrange("p (s f) -> p s f", s=2)
            seld8_v = seld8_sb.rearrange("p (s t) -> p s t", s=2)
            sell8_v = sell8_sb.rearrange("p (s t) -> p s t", s=2)
            sinksel_v = sinksel_sb.rearrange("o (s f) -> o s f", s=2)
            sinkrow_v = sinkrow_sb.rearrange("o (s q) -> o s q", s=2)

            att_ctx = [
                tc.tile_pool(name="wexp", bufs=8),
                tc.tile_pool(name="dnp", bufs=4),
                tc.tile_pool(name="rbp", bufs=3),
            ]
            wexp, dnp, rbp = [c.__enter__() for c in att_ctx]
            wo_tiles = {}
            xtp_ctx = [
                tc.tile_pool(name="xtp", bufs=1),
                tc.tile_pool(name="wqp", bufs=4),
                tc.tile_pool(name="ropep", bufs=2),
            ]
            xtp, wqp, ropep = [c.__enter__() for c in xtp_ctx]
            if True:
                x8_sb = xtp.tile([128, KTP * 2 * NT], FP8)
                x8_v = x8_sb.rearrange("p (k s t) -> p k s t", k=KTP, s=2)
                wqc8_sb = xtp.tile([128, KTP * 1536], FP8)
                wqc8_v = wqc8_sb.rearrange("p (k j s f) -> p k j s f",
                                           k=KTP, j=6, s=2)
                wq_tiles = {}

                def wq_dma(m, gate_col=None):
                    t = wqp.tile([128, KTP * 256], FP8, tag="wq",
                                 name=f"wqm{m}")
                    if gate_col is not None:
                        gate(t, gate_col)
                    nc.sync.dma_start(out=t, in_=wq8_d[m - 4, :, :])
                    wq_tiles[m] = t

                # --- DMA issue: x8 lo+hi stream + wqc8 chase chunks on sync
                # queue; ktile-granular at the front so the chase starts
                # early, batched at the back.
                def x_dma(lo, hi):
                    nc.sync.dma_start(
                        out=x8_sb[:, lo * 2 * NT:hi * 2 * NT],
                        in_=x8_d[:, lo * 2 * NT:hi * 2 * NT])

                def wc_dma(lo, hi):
                    nc.sync.dma_start(out=wqc8_sb[:, lo * 1536:hi * 1536],
                                      in_=wqc8_d[:, lo * 1536:hi * 1536])

                # ktile arrival order: wqc8[k] then x[k] (lo+hi together)
                for lo, hi in ((0, 1), (1, 2), (2, 3), (3, 4)):
                    wc_dma(lo, hi)
                    x_dma(lo, hi)
                for lo, hi in ((4, 6), (6, 8), (8, 10)):
                    wc_dma(lo, hi)
                    x_dma(lo, hi)

                def gate(sb, col):
                    # 1-element copy whose input arrives late: keeps the
                    # subsequent DMA from being hoisted ahead of the stream
                    nc.vector.tensor_copy(out=sb[0:1, 0:1],
                                          in_=x8_sb[0:1, col:col + 1])

                for sb in (cos_sb, sin_sb, bq_sb):
                    gate(sb, 9 * NT)
                nc.sync.dma_start(out=cos_sb, in_=cos_d[:, :])
                nc.sync.dma_start(out=sin_sb, in_=sin_d[:, :])
                nc.sync.dma_start(out=bq_sb, in_=bqkv_d[:, :])
                for lo, hi in ((10, 12), (12, 15), (15, 18)):
                    wc_dma(lo, hi)
                    x_dma(lo, hi)
                for sb in (mask8_sb, seld8_sb, sell8_sb, sinksel_sb,
                           sinkrow_sb, id_sb):
                    gate(sb, 14 * NT)
                nc.sync.dma_start(out=mask8_sb, in_=mask8_d[:, :])
                nc.sync.dma_start(out=seld8_sb, in_=seld8_d[:, :])
                nc.sync.dma_start(out=sell8_sb, in_=sell8_d[:, :])
                nc.sync.dma_start(out=sinksel_sb, in_=sinksel_d[:, :])
                nc.sync.dma_start(out=sinkrow_sb, in_=sinkrow_d[:, :])
                nc.sync.dma_start(out=id_sb, in_=id_d[:, :])
                for lo, hi in ((18, 21), (21, KTP)):
                    wc_dma(lo, hi)
                    x_dma(lo, hi)
                for m in (4, 5, 6, 7):
                    wq_dma(m, 2 * KTP * NT - 8 * NT + m * NT)
                for t in range(16):
                    nc.vector.memset(vt_sb[:, t * 65 + 64:t * 65 + 65], 1.0)

                def qkv_dest(m):
                    if m == 8:
                        return k_sb
                    if m == 9:
                        return v_sb
                    return q_sb[:, m * NT:(m + 1) * NT]

                def qkv_epilogue(m, ps_cs):
                    dest = qkv_dest(m)
                    for c, ps in enumerate(ps_cs):
                        nc.scalar.activation(
                            out=dest[:, c * CH:(c + 1) * CH], in_=ps,
                            func=mybir.ActivationFunctionType.Identity,
                            bias=bq_sb[:, m:m + 1], scale=1.0 / 64.0,
                        )

                def rope(dest, c=None):
                    cs = (0, 1) if c is None else (c,)
                    for cc in cs:
                        sl = slice(cc * CH, (cc + 1) * CH)
                        sh = ropep.tile([128, CH], BF16, tag="rope")
                        nc.vector.stream_shuffle(out=sh, in_=dest[:, sl],
                                                 mask=PAIR_SWAP)
                        nc.vector.tensor_mul(sh, sh, sin_sb[:, sl])
                        nc.vector.tensor_mul(dest[:, sl], dest[:, sl],
                                             cos_sb[:, sl])
                        nc.vector.tensor_add(dest[:, sl], dest[:, sl], sh)

                # --- chase: m8/m9/m0/m1 hi-lo DR chains follow the x stream.
                # Accumulation chain per (j, c): cross_0 .. cross_k .. with
                # hihi(i) after odd k; stop on the last hihi.
                kv_ctx = tc.tile_pool(name="ps_kv", bufs=2, space="PSUM")
                ps_kv = kv_ctx.__enter__()
                ps8 = ps_kv.tile([128, NT], F32, tag="kv", name="mm8")
                ps9 = ps_kv.tile([128, NT], F32, tag="kv", name="mm9")
                warm = xtp.tile([128, CH], BF16)
                nc.gpsimd.memset(warm, 0.0)
                warm_ctx = tc.tile_pool(name="ps_warm", bufs=1, space="PSUM")
                ps_warm_pool = warm_ctx.__enter__()
                ps_warm = ps_warm_pool.tile([128, CH], F32)
                for i in range(8):
                    nc.tensor.matmul(
                        ps_warm, warm[:, 0:128], warm,
                        start=(i == 0), stop=(i == 7),
                    )
                warm_ctx.__exit__(None, None, None)
                mm_ctx = tc.tile_pool(name="ps_mm", bufs=4, space="PSUM",
                                      side="right")
                ps_mm = mm_ctx.__enter__()
                ps03 = {m: ps_mm.tile([128, CH], F32, tag="mm",
                                      name=f"mm{m}c0")
                        for m in (0, 1, 2, 3)}

                def chase_mm(j, k, c, out):
                    mv_c = x8_v[:, k, :, c * CH:(c + 1) * CH]
                    if k < KT:
                        # cross: (w8hi_k x x8lo_k) + (w8lo_k x x8hi_k)
                        nc.tensor.matmul(
                            out, wqc8_v[:, k, j, :, :], mv_c,
                            start=(k == 0), stop=False, perf_mode=DR,
                        )
                    if k % 2 == 1:
                        # hi-hi pair (k-1, k)
                        nc.tensor.matmul(
                            out,
                            wqc8_v[:, k - 1:k + 1, j, 0, :],
                            x8_v[:, k - 1:k + 1, 1, c * CH:(c + 1) * CH],
                            start=False, stop=(k == KTP - 1), perf_mode=DR,
                        )

                for k in range(KTP):
                    for j in range(6):
                        m = CHASE_M[j]
                        if m == 8:
                            for c in range(2):
                                chase_mm(j, k, c, ps8[:, c * CH:(c + 1) * CH])
                        elif m == 9:
                            for c in range(2):
                                chase_mm(j, k, c, ps9[:, c * CH:(c + 1) * CH])
                        else:
                            chase_mm(j, k, 0, ps03[m])

                qkv_epilogue(8, (ps8[:, 0:CH], ps8[:, CH:NT]))
                rope(k_sb)
                qkv_epilogue(9, (ps9[:, 0:CH], ps9[:, CH:NT]))
                kv_ctx.__exit__(None, None, None)
                vt_ctx = tc.tile_pool(name="ps_vt", bufs=2, space="PSUM")
                ps_vt = vt_ctx.__enter__()
                for g in range(2):
                    for kt in range(8):
                        pst = ps_vt.tile([128, 64], BF16, tag="vt")
                        nc.tensor.matmul(
                            pst,
                            v_sb[g * 64:(g + 1) * 64, kt * 128:(kt + 1) * 128],
                            id_sb[g * 64:(g + 1) * 64, :],
                            is_transpose=True,
                            start=True, stop=True,
                        )
                        nc.vector.tensor_copy(
                            out=vt_sb[:, (g * 8 + kt) * 65:
                                      (g * 8 + kt) * 65 + 64],
                            in_=pst,
                        )
                vt_ctx.__exit__(None, None, None)

                # chunk-1 chains for the chase m0..m3 (x8 fully resident)
                def qkv_c1(m, j):
                    ps = ps_mm.tile([128, CH], F32, tag="mm",
                                    name=f"mm{m}c1")
                    for k in range(KTP):
                        chase_mm(j, k, 1, ps)
                    return ps

                def qkv_mms(m, wq_v, inject=False):
                    # wq_v: [128, k, s, f] view of this m's weight tile
                    ps_cs = []
                    for c in range(2):
                        ps = ps_mm.tile([128, CH], F32, tag="mm",
                                        name=f"mm{m}c{c}")
                        ps_cs.append(ps)
                        for k in range(KTP):
                            mv_c = x8_v[:, k, :, c * CH:(c + 1) * CH]
                            if k < KT:
                                nc.tensor.matmul(
                                    ps, wq_v[:, k, :, :], mv_c,
                                    start=(k == 0), stop=False, perf_mode=DR,
                                )
                            if k % 2 == 1:
                                nc.tensor.matmul(
                                    ps,
                                    wq_v[:, k - 1:k + 1, 0, :],
                                    x8_v[:, k - 1:k + 1, 1,
                                         c * CH:(c + 1) * CH],
                                    start=False, stop=(k == KTP - 1),
                                    perf_mode=DR,
                                )
                            if inject:
                                att_step()
                    return ps_cs

                def q_tile(m, inject=False):
                    wq_v = wq_tiles.pop(m).rearrange(
                        "p (k s f) -> p k s f", k=KTP, s=2)
                    ps_cs = qkv_mms(m, wq_v, inject)
                    dest = qkv_dest(m)
                    for c, ps in enumerate(ps_cs):
                        nc.scalar.activation(
                            out=dest[:, c * CH:(c + 1) * CH], in_=ps,
                            func=mybir.ActivationFunctionType.Identity,
                            bias=bq_sb[:, m:m + 1], scale=1.0 / 64.0,
                        )
                        rope(dest, c)

                for m in range(4):
                    psc1 = qkv_c1(m, 2 + m)
                    for c, ps in ((0, ps03[m]), (1, psc1)):
                        nc.scalar.activation(
                            out=qkv_dest(m)[:, c * CH:(c + 1) * CH], in_=ps,
                            func=mybir.ActivationFunctionType.Identity,
                            bias=bq_sb[:, m:m + 1], scale=1.0 / 64.0,
                        )
                        rope(qkv_dest(m), c)

                # --- attention machinery
                att1_ctx = [
                    tc.tile_pool(name="ps_att", bufs=2, space="PSUM"),
                    tc.tile_pool(name="ps_pv", bufs=2, space="PSUM"),
                ]
                cur_pools = [[c.__enter__() for c in att1_ctx]]

                def attn_A(qt, a, g):
                    ps_att, ps_pv = cur_pools[0]
                    prng = slice(g * 64, (g + 1) * 64)
                    kts = [qt] if qt == 0 else [qt - 1, qt]
                    rhs_q = q_v[prng, 4 * a:4 * a + 4, qt * 128:(qt + 1) * 128]
                    ws = []
                    for kt in kts:
                        psl = ps_att.tile([128, CH], F32, tag="l")
                        sel = seld8_v if kt == qt else sell8_v
                        nc.tensor.matmul(
                            psl, mask8_v[:, :, :], sel[:, :, :],
                            start=True, stop=False, perf_mode=DR,
                        )
                        nc.tensor.matmul(
                            psl,
                            k_sb[prng, kt * 128:(kt + 1) * 128],
                            rhs_q,
                            start=False, stop=True,
                        )
                        w = wexp.tile([128, CH], BF16, tag="w")
                        nc.scalar.activation(
                            out=w, in_=psl,
                            func=mybir.ActivationFunctionType.Exp,
                            scale=0.125,
                        )
                        ws.append((kt, w))
                    return (qt, a, g, ws)

                def attn_B(st):
                    ps_att, ps_pv = cur_pools[0]
                    qt, a, g, ws = st
                    prng = slice(g * 64, (g + 1) * 64)
                    pspv = ps_pv.tile([65, CH], F32, tag="pv")
                    so = (2 * g + a) * CH
                    nc.tensor.matmul(
                        pspv, sinksel_v[:, :, 0:65],
                        sinkrow_v[:, :, so:so + CH],
                        start=True, stop=False, perf_mode=DR,
                    )
                    for i, (kt, w) in enumerate(ws):
                        nc.tensor.matmul(
                            pspv,
                            vt_sb[:, (g * 8 + kt) * 65:(g * 8 + kt + 1) * 65],
                            w,
                            start=False,
                            stop=(i == len(ws) - 1),
                        )
                    dn = dnp.tile([1, CH], F32, tag="dn")
                    dnb = dnp.tile([64, CH], F32, tag="dnb")
                    nc.vector.reciprocal(out=dn, in_=pspv[64:65, :])
                    nc.gpsimd.partition_broadcast(dnb, dn)
                    nc.vector.tensor_tensor(
                        out=a_v[prng, 4 * a:4 * a + 4,
                                qt * 128:(qt + 1) * 128],
                        in0=pspv[0:64, :],
                        in1=dnb,
                        op=mult,
                    )

                def attn_C(qt, a):
                    # hi/lo fp8 extraction for outproj over both g at once
                    src = a_v[:, 4 * a:4 * a + 4, qt * 128:(qt + 1) * 128]
                    hi = a8_v[:, 1, 4 * a:4 * a + 4, qt * 128:(qt + 1) * 128]
                    lo = a8_v[:, 0, 4 * a:4 * a + 4, qt * 128:(qt + 1) * 128]
                    nc.vector.tensor_copy(out=hi, in_=src)
                    rb = rbp.tile([128, CH], BF16, tag="rb")
                    rb_v = rb.rearrange("p (h t) -> p h t", h=4)
                    nc.vector.tensor_tensor(out=rb_v, in0=src, in1=hi, op=sub)
                    nc.scalar.activation(
                        out=lo, in_=rb_v,
                        func=mybir.ActivationFunctionType.Copy, scale=8.0)

                from collections import deque
                att_pending = deque()
                att_inflight = deque()
                attC_pending = deque()
                att_done = {}

                def att_post(qt, a, g):
                    att_done[(qt, a, g)] = True
                    if att_done.get((qt, a, 1 - g)):
                        attC_pending.append((qt, a))

                extracted = set()

                def att_step():
                    if attC_pending:
                        qa = attC_pending.popleft()
                        attn_C(*qa)
                        extracted.add(qa)
                    elif len(att_inflight) >= 4 or (not att_pending
                                                    and att_inflight):
                        st = att_inflight.popleft()
                        attn_B(st)
                        att_post(st[0], st[1], st[2])
                    elif att_pending:
                        att_inflight.append(attn_A(*att_pending.popleft()))

                def ensure_extracted(qts, a):
                    need = {(qt, a) for qt in qts}
                    for _ in range(200):
                        if need <= extracted:
                            return
                        att_step()
                    raise RuntimeError(f"cannot extract {need}")

                def att_drain(with_c=False):
                    while att_pending or att_inflight:
                        att_step()
                    if with_c:
                        while attC_pending:
                            attn_C(*attC_pending.popleft())

                # W1: Q m4..m7 with a=0 attention groups pipelined
                att_pending.extend((qt, 0, g) for qt in range(8)
                                   for g in range(2))
                for m in range(4, 8):
                    q_tile(m, inject=True)
                att_drain()
                for c in reversed(att1_ctx):
                    c.__exit__(None, None, None)
                mm_ctx.__exit__(None, None, None)
                for c in reversed(xtp_ctx):
                    c.__exit__(None, None, None)
                att2_ctx = [
                    tc.tile_pool(name="ps_att2", bufs=2, space="PSUM"),
                    tc.tile_pool(name="ps_pv2", bufs=4, space="PSUM"),
                ]
                cur_pools[0] = [c.__enter__() for c in att2_ctx]

                # W2: a=1 qt0-3 + extractions; must fully drain before the
                # chunk-0 projection reads their outputs
                att_pending.extend((qt, 1, g) for qt in range(4)
                                   for g in range(2))
                for _ in range(16):
                    att_step()
                att_drain(with_c=True)

                # W3/W4: output projection; chunk 0 interleaves with the
                # remaining attention groups (a=1, qt 4..7)
                with (
                    tc.tile_pool(name="wop", bufs=23) as wop,
                    tc.tile_pool(name="otp", bufs=17) as otp,
                    tc.tile_pool(name="ps_o", bufs=2, space="PSUM") as ps_o,
                ):
                    def wo_preload(m):
                        t = wop.tile([128, 3 * OUT_K * 128], FP8,
                                     tag="wo", name=f"wo{m}")
                        nc.scalar.dma_start(out=t, in_=wo8_d[m, :, :])
                        wo_tiles[m] = t

                    for _m in range(3):
                        wo_preload(_m)

                    ot_tiles = {}

                    def outproj_m(cs, m, inject=False, release=True,
                                  preload=None, keep_ot=False):
                        if (preload is not None and preload < OUT_M
                                and preload not in wo_tiles):
                            wo_preload(preload)
                        if inject:
                            att_step()
                            att_step()
                        wo_sb = wo_tiles.pop(m) if release else wo_tiles[m]
                        wo_v = wo_sb.rearrange("p (b k f) -> p b k f",
                                               b=3, k=OUT_K)
                        if m in ot_tiles:
                            ot = ot_tiles.pop(m)
                        else:
                            ot = otp.tile([128, NT], F16, tag="ot",
                                          name=f"ot{m}")
                        if keep_ot:
                            ot_tiles[m] = ot
                        for c in cs:
                            ps = ps_o.tile([128, CH], F32, tag="o",
                                           name=f"o{c}_{m}")
                            csl = slice(c * CH, (c + 1) * CH)
                            first = True
                            for i in range(OUT_K // 2):
                                nc.tensor.matmul(
                                    ps, wo_v[:, 0, 2 * i:2 * i + 2, :],
                                    a8_v[:, 1, 2 * i:2 * i + 2, csl],
                                    start=first, stop=False, perf_mode=DR,
                                )
                                first = False
                                if inject and i % 2 == 1:
                                    att_step()
                            for k in range(OUT_K):
                                # cross: (w8hiD8_k x a8lo_k)+(w8lo_k x a8hi_k)
                                nc.tensor.matmul(
                                    ps, wo_v[:, 1:3, k, :],
                                    a8_v[:, :, k, csl],
                                    start=False, stop=(k == OUT_K - 1),
                                    perf_mode=DR,
                                )
                                if inject and k % 2 == 1:
                                    att_step()
                            nc.vector.tensor_scalar(
                                out=ot[:, csl], in0=ps,
                                scalar1=1.0 / 32.0, scalar2=None,
                                op0=mult)
                        if keep_ot:
                            pass
                        elif len(cs) == 2 or m in ot_full:
                            nc.sync.dma_start(out=out_d[m, :, :], in_=ot)
                        else:
                            c = cs[0]
                            nc.sync.dma_start(
                                out=out_d[m, :, c * CH:(c + 1) * CH],
                                in_=ot[:, c * CH:(c + 1) * CH])

                    att_pending.extend((qt, 1, g) for qt in range(4, 8)
                                       for g in range(2))
                    att_step()
                    att_step()
                    att_step()
                    ot_full = set(range(8, OUT_M))
                    for m in range(OUT_M):
                        outproj_m((0,), m, inject=(m < 14), release=False,
                                  preload=m + 3, keep_ot=(m >= 8))
                    att_drain(with_c=True)
                    for m in range(8, OUT_M):
                        outproj_m((1,), m)
                    for m in range(7):
                        outproj_m((1,), m)
                    # final tile: two half-width chains; epilogues split
                    # across DVE/ACT, stores on two queues
                    wo_sb = wo_tiles.pop(7)
                    wo_v = wo_sb.rearrange("p (b k f) -> p b k f",
                                           b=3, k=OUT_K)
                    for h, (epi, q) in enumerate(
                            (("dve", nc.sync), ("act", nc.scalar))):
                        ps = ps_o.tile([128, 256], F32, tag="o",
                                       name=f"ofin{h}")
                        csl = slice(CH + h * 256, CH + (h + 1) * 256)
                        first = True
                        for i in range(OUT_K // 2):
                            nc.tensor.matmul(
                                ps, wo_v[:, 0, 2 * i:2 * i + 2, :],
                                a8_v[:, 1, 2 * i:2 * i + 2, csl],
                                start=first, stop=False, perf_mode=DR,
                            )
                            first = False
                        for k in range(OUT_K):
                            nc.tensor.matmul(
                                ps, wo_v[:, 1:3, k, :],
                                a8_v[:, :, k, csl],
                                start=False, stop=(k == OUT_K - 1),
                                perf_mode=DR,
                            )
                        ot = otp.tile([128, 256], F16, tag="otf")
                        if epi == "dve":
                            nc.vector.tensor_scalar(
                                out=ot, in0=ps, scalar1=1.0 / 32.0,
                                scalar2=None, op0=mult)
                        else:
                            nc.scalar.activation(
                                out=ot, in_=ps,
                                func=mybir.ActivationFunctionType.Copy,
                                scale=1.0 / 32.0)
                        q.dma_start(out=out_d[7, :, csl], in_=ot)

                for c in reversed(att2_ctx):
                    c.__exit__(None, None, None)
                for c in reversed(att_ctx):
                    c.__exit__(None, None, None)

        if timing_mode:
            with tc.tile_pool(name="dummyp", bufs=1) as dummyp:
                dt_sb = dummyp.tile([1, 2], F32)
                nc.vector.memset(dt_sb, 1.0)
                nc.sync.dma_start(out=dummy_d[:, :], in_=dt_sb)

    nc.compile()
    return nc


# ---------------------------------------------------------------- host prep
def _rope_tables():
    # verbatim fp32 port of the reference YaRN cache
    steps = np.arange(0, 64, 2, dtype=np.float32)
    freq = np.power(np.float32(150000.0), steps / np.float32(64))
    conc = np.float32(0.1) * np.log(np.float32(32.0)) + 1.0
    d_half = np.float32(32.0)
    log_base = np.log(np.float32(150000.0))
    low = d_half * np.log(np.float32(4096) / (np.float32(32.0) * np.float32(2.0 * np.pi))) / log_base
    high = d_half * np.log(np.float32(4096) / (np.float32(1.0) * np.float32(2.0 * np.pi))) / log_base
    ramp = (np.arange(32, dtype=np.float32) - low) / (high - low)
    mask = 1.0 - np.clip(ramp, 0.0, 1.0)
    inv_freq = (1.0 / (np.float32(32.0) * freq)) * (1.0 - mask) + (1.0 / freq) * mask
    pos = np.arange(SEQ, dtype=np.float32)
    freqs = np.einsum("i,j->ij", pos, inv_freq.astype(np.float32))
    cos = (np.cos(freqs) * conc).astype(np.float32)  # (SEQ, 32)
    sin = (np.sin(freqs) * conc).astype(np.float32)
    return cos, sin


F8NP = ml_dtypes.float8_e4m3


def _bf16(a):
    return np.ascontiguousarray(a.astype(ml_dtypes.bfloat16))


def _hilo8(a):
    """fp8 residual split: a ~= hi + lo (same scale)."""
    a = np.asarray(a, np.float32)
    hi = a.astype(F8NP)
    lo = (a - hi.astype(np.float32)).astype(F8NP)
    return hi, lo


_ILV = np.empty(64, np.int64)
_ILV[0::2] = np.arange(32)
_ILV[1::2] = np.arange(32) + 32


def prep_inputs(x, norm_w, qkv_w, qkv_b, out_w, sinks):
    x = np.asarray(x, np.float32)
    norm_w = np.asarray(norm_w, np.float32)
    qkv_w = np.asarray(qkv_w, np.float32)
    qkv_b = np.asarray(qkv_b, np.float32)
    out_w = np.asarray(out_w, np.float32)
    sinks = np.asarray(sinks, np.float32)

    # host RMSNorm (norm_w folded into qkv_w below)
    rms = np.mean(x * x, axis=-1, keepdims=True, dtype=np.float32)
    xn = x * (1.0 / np.sqrt(rms + np.float32(EPS)))

    cos, sin = _rope_tables()
    cosT, sinT = cos.T, sin.T                      # (32, SEQ)
    cos64 = np.repeat(cosT, 2, axis=0)             # lo/hi both use cos_i
    sin64 = np.repeat(sinT, 2, axis=0).copy()
    sin64[0::2] *= -1.0                            # lo gets -sin
    cos128 = _bf16(np.concatenate([cos64, cos64], axis=0))
    sin128 = _bf16(np.concatenate([sin64, sin64], axis=0))

    i = np.arange(128)[:, None]
    j = np.arange(128)[None, :]
    maskd = np.where(i <= j, 0.0, MASK_NEG).astype(np.float32)   # diag block
    maskl = np.where(i > j, 0.0, MASK_NEG).astype(np.float32)    # low block
    # stationary [maskd.T | maskl.T]; moving selectors pick the slot
    mask8 = np.ascontiguousarray(
        np.concatenate([maskd.T, maskl.T], axis=1)).astype(F8NP)
    eye_t = np.tile(np.eye(128, dtype=np.float32) * 8.0, (1, 4))
    z = np.zeros_like(eye_t)
    seld8 = np.ascontiguousarray(
        np.concatenate([eye_t, z], axis=1)).astype(F8NP)
    sell8 = np.ascontiguousarray(
        np.concatenate([z, eye_t], axis=1)).astype(F8NP)
    sinksel = np.zeros((1, 160), np.float32)
    sinksel[0, 64] = 1.0
    sinksel[0, 80 + 64] = 1.0
    sinksel8 = sinksel.astype(F8NP)
    eye = np.eye(64, dtype=np.float32)
    ident64 = _bf16(np.concatenate([eye, eye], axis=0))  # (128, 64)

    w_eff = (qkv_w * norm_w[None, :]) * 64.0
    b_eff = qkv_b.copy()

    in_maps = []
    for c in range(8):
        b, g2 = divmod(c, 4)
        # Q m-tile m holds heads (16*g2+m) [partitions 0:64] and (16*g2+8+m)
        # [partitions 64:128], rope-pair interleaved within each head.
        qheads = np.empty(16, np.int64)
        qheads[0::2] = 16 * g2 + np.arange(8)        # g=0 heads, even slots
        qheads[1::2] = 16 * g2 + 8 + np.arange(8)    # g=1 heads, odd slots
        qrows = (qheads[:, None] * D + _ILV[None, :]).reshape(-1)
        krows = NH * D + np.arange(2 * g2 * D, 2 * (g2 + 1) * D)
        vrows = (NH + NKV) * D + np.arange(2 * g2 * D, 2 * (g2 + 1) * D)
        krows = krows.reshape(2, 64)[:, _ILV].reshape(-1)
        rowsel = np.concatenate([qrows, krows, vrows])
        Wc = w_eff[rowsel]                          # (1280, 2880)
        bc = b_eff[rowsel]

        WcT = np.zeros((HIDP2, 1280), np.float32)
        WcT[:HID] = Wc.T
        whi, wlo = _hilo8(WcT)                      # (3072, 1280)
        # per m: [128, KTP, 2, 128]  (k, s=hi/lo, f)
        wq_all = np.stack([whi, wlo]).reshape(2, KTP, 128, QKV_M, 128)
        wq_all = np.ascontiguousarray(
            wq_all.transpose(3, 2, 1, 0, 4))        # (m, p, k, s, f)
        # chase tensor: col = k*1536 + j*256 + s*128 + f
        wqc8 = np.ascontiguousarray(
            wq_all[list(CHASE_M)]                   # (6, 128, KTP, 2, 128)
            .transpose(1, 2, 0, 3, 4).reshape(128, KTP * 1536))
        wq8 = np.ascontiguousarray(
            wq_all[4:8].reshape(4, 128, KTP * 256))
        bqkv = np.ascontiguousarray(bc.reshape(QKV_M, 128).T)

        # x: [lo | hi] blocks, ktile-major
        xp = np.zeros((HIDP2, NT), np.float32)
        xp[:HID] = xn[b].T
        xhi, xlo = _hilo8(xp)
        x8 = np.stack([xlo, xhi]).reshape(2, KTP, 128, NT)
        x8 = np.ascontiguousarray(
            x8.transpose(2, 1, 0, 3).reshape(128, KTP * 2 * NT))

        # attn feature f: tile ft=f//128, partition p=f%128 -> g=p//64, hq=ft
        f = np.arange(1024)
        colsel = (16 * g2 + 8 * ((f % 128) // 64) + f // 128) * D + (f % 64)
        WoT = np.zeros((1024, HIDP), np.float32)
        WoT[:, :HID] = out_w[:, colsel].T * 32.0
        wohi, wolo = _hilo8(WoT)
        woD8 = (WoT / 8.0).astype(F8NP)
        # per m: [128, 3, OUT_K, 128] -> [hi | hiD8 | lo]
        wo_all = np.stack([wohi, woD8, wolo])       # (3, 1024, HIDP)
        wo_all = wo_all.reshape(3, OUT_K, 128, OUT_M, 128)
        wo8 = np.ascontiguousarray(
            wo_all.transpose(3, 2, 0, 1, 4).reshape(OUT_M, 128, 3 * OUT_K * 128))

        sinkrow = np.empty((1, 4 * CH), np.float32)
        for g in range(2):
            for a in range(2):
                hl = 8 * g + 4 * a + np.arange(4)        # local heads per quad
                se = np.exp(sinks[16 * g2 + hl].astype(np.float32))
                sinkrow[0, (2 * g + a) * CH:(2 * g + a + 1) * CH] = \
                    np.repeat(se, 128)
        shi, slo = _hilo8(sinkrow)
        sinkrow8 = np.ascontiguousarray(
            np.concatenate([shi, slo], axis=1))      # [1, 2*2048]

        in_maps.append({
            "x8": x8, "wqc8": wqc8, "wq8": wq8, "bqkv": bqkv,
            "wo8": wo8,
            "cos128": cos128, "sin128": sin128,
            "mask8": mask8, "seld8": seld8, "sell8": sell8,
            "sinksel8": sinksel8, "sinkrow8": sinkrow8,
            "ident64": ident64,
        })
    return in_maps


def unshard(results, x, out_b):
    x = np.asarray(x, np.float32)
    out_b = np.asarray(out_b, np.float32)
    y = np.empty((B, SEQ, HID), np.float32)
    for b in range(B):
        acc = np.zeros((HIDP, NT), np.float64)
        for g2 in range(4):
            acc += results[4 * b + g2]["out_t"].astype(np.float64).reshape(HIDP, NT)
        y[b] = x[b] + acc[:HID].T.astype(np.float32) + out_b[None, :]
    return y


_NC_CACHE = []


def kernel(x, norm_w, qkv_w, qkv_b, out_w, out_b, sinks):
    in_maps = prep_inputs(x, norm_w, qkv_w, qkv_b, out_w, sinks)
    if not _NC_CACHE:
        _NC_CACHE.append(build_nc())
    nc = _NC_CACHE[0]
    res = run_bass_kernel_spmd(nc, in_maps, core_ids=list(range(8)))
    return unshard(res.results, x, out_b)
